# revision 52
# baseline (speedup 1.0000x reference)
"""Trainium2 Bass kernel for causal multi-head attention.

Problem: x[1,4096,1024] -> MHA(16 heads, head_dim 64, causal) -> out[1,4096,1024]
  q,k,v = x @ W_{q,k,v}; scores = q k^T / 8 (causal); out = softmax(scores) v @ W_o + b_o

Sharding: tensor-parallel over heads, 2 heads (128 feature dims) per core.
Each core computes a full-width partial output ctx_c @ W_o[slice_c] which the
host sums over the 8 cores (row-parallel out-projection).

Design:
- The attention inner loop (scores -> exp -> P@V) is ACT-paced; other
  engines' work is software-pipelined INTO it as "filler" thunks, because
  engine queues are strict FIFO: if PE's next instruction depends on a
  pending exp, PE idles and the HAM clock gate drops it to 1.2 GHz.
  Fillers = next chunk's QKV-projection matmuls (phase1) + previous
  chunks' out-projections (kept in a carry-over reserve so the ACT-heavy
  late chunks still get PE work). P@V for k-tile kt is emitted two
  iterations behind scores(kt) so exp(kt) latency is hidden.
- A dependency-free burst of warmup matmuls fills the startup DMA wait so
  the PE enters the pipeline at 2.4 GHz (the schedule is otherwise
  bistable between a ~255us warm mode and a ~305us cold-oscillating one).
- Scores for both heads of one 128-k-tile go in one [128, 2(h), 512] fp32
  PSUM tile (one bank per head; the tile_position pair runs concurrently);
  one ACTIVATE(exp) covers both heads and applies the 1/8 softmax scale
  via its free affine input.
- Diagonal score tiles are column-restricted (fully-masked columns never
  touch any engine); the in-block causal triangle is zeroed by one
  [128, 2, 128] bf16 DVE multiply.
- ctx is normalized BEFORE the out-projection: per-q reciprocals of the
  softmax sums (picked up by an appended ones-column in P@V) via
  reciprocal_approx_fast, broadcast to a [64, 512] matrix by a PE
  outer-product, fused into the DVE eviction of ctx^T. The out-projection
  is then one contraction-128 matmul per output block with no DVE math.
- x and the QKV weights are host-pre-permuted so every DMA line is
  contiguous per partition; the whole x is prefetched up front across the
  sync/gpsimd queues. Output partials are bf16; the host sums in fp64.

kernel(**inputs) takes the FULL unsharded inputs and returns the FULL output.
"""

import sys

import numpy as np

for _p in ("/opt/trn_rl_repo", "/root/.axon_site/_ro/trn_rl_repo"):
    if _p not in sys.path:
        try:
            import concourse  # noqa: F401

            break
        except ImportError:
            sys.path.insert(0, _p)

N_CORES = 8
SEQ = 4096
D = 1024
DC = 128  # per-core slice of the head dim (2 heads x 64)
HD = 64


def build_bass(n=SEQ, d=D):
    """Trace the per-core SPMD Bass program. n = sequence length."""
    import concourse.bacc as bacc
    import concourse.mybir as mybir
    import concourse.tile as tile
    from concourse.masks import make_identity

    fp32 = mybir.dt.float32
    fp16 = mybir.dt.float16
    bf16 = mybir.dt.bfloat16
    Exp = mybir.ActivationFunctionType.Exp

    assert n % 512 == 0 and d % 128 == 0
    NT = n // 128  # 128-row seq tiles
    NCH = n // 512  # 512-col seq chunks
    DIT = d // 128  # input-dim 128-tiles
    SCALE = 1.0 / float(np.sqrt(HD))

    nc = bacc.Bacc("TRN2", target_bir_lowering=False)

    # host-pre-permuted layouts so every DMA line is contiguous per
    # partition: x2[p, ch, dit, c], w*2[p, dit, c]
    x2_d = nc.dram_tensor("x2", (128, NCH, DIT, 512), bf16, kind="ExternalInput")
    wq_d = nc.dram_tensor("wq", (128, DIT, DC), bf16, kind="ExternalInput")
    wk_d = nc.dram_tensor("wk", (128, DIT, DC), bf16, kind="ExternalInput")
    wv_d = nc.dram_tensor("wv", (128, DIT, DC), bf16, kind="ExternalInput")
    wo_d = nc.dram_tensor("wo", (DC, d), bf16, kind="ExternalInput")
    out_d = nc.dram_tensor("out", (n, d), bf16, kind="ExternalOutput")

    with tile.TileContext(nc) as tc:
        with (
            tc.tile_pool(name="const", bufs=1) as const_pool,
            tc.tile_pool(name="weights", bufs=1) as w_pool,
            tc.tile_pool(name="big", bufs=1) as big_pool,
            tc.tile_pool(name="xin", bufs=2) as xin_pool,
            tc.tile_pool(name="vt", bufs=2) as vt_pool,
            tc.tile_pool(name="pm", bufs=4) as pm_pool,
            tc.tile_pool(name="small", bufs=2) as s_pool,
            tc.tile_pool(name="outsb", bufs=3) as out_pool,
            tc.tile_pool(name="ps", bufs=1, space="PSUM") as ps,
        ):
            # Dependency-free warmup matmuls, first thing: back-to-back PE
            # work so the HAM clock gate reaches 8/8 before the pipeline
            # starts, covering the startup DMA wait (the schedule is
            # bistable: a ~2.7us PE idle seam between warmup end and the
            # wq/xch0 DMA landing sends the whole run into a cold
            # ~305us mode). DVE memset so warmup isn't queued behind the
            # gpsimd constant setup.
            scr_sb = const_pool.tile([128, 512], bf16)
            nc.vector.memset(scr_sb[:], 0.0)
            for _ in range(28):
                scr_ps = ps.tile(
                    [128, 512], fp32, tag="w", bufs=2, name="scr_ps"
                )
                nc.tensor.matmul(
                    scr_ps[:], scr_sb[:, 0:128], scr_sb[:], start=True, stop=True
                )

            # ---- x chunk 0 first on the sync queue so it lands before
            # the warmup ends (it gates the first real matmul) ----
            xchs = []
            for ch in range(NCH):
                xchs.append(
                    xin_pool.tile(
                        [128, DIT, 512], bf16, tag=f"xch{ch}", bufs=1, name="xch"
                    )
                )
            nc.sync.dma_start(xchs[0][:], x2_d[:, 0, :, :])

            # ---- constants ----
            ident = const_pool.tile([128, 128], bf16)
            make_identity(nc, ident[:])
            # tri2[kl, h, q'] = 1 if q' >= kl else 0 -- the within-block
            # causal triangle, replicated over the two heads.
            tri2 = const_pool.tile([128, 2, 128], bf16)
            nc.gpsimd.memset(tri2[:], 1.0)
            for h in range(2):
                nc.gpsimd.affine_select(
                    out=tri2[:, h, :],
                    in_=tri2[:, h, :],
                    compare_op=mybir.AluOpType.is_ge,
                    fill=0.0,
                    base=0,
                    pattern=[[1, 128]],
                    channel_multiplier=-1,
                )
            ones64 = const_pool.tile([1, HD], fp32)
            nc.gpsimd.memset(ones64[:], 1.0)

            # ---- weights ----
            wq_sb = w_pool.tile([128, DIT, DC], bf16)
            wk_sb = w_pool.tile([128, DIT, DC], bf16)
            wv_sb = w_pool.tile([128, DIT, DC], bf16)
            nc.scalar.dma_start(wq_sb[:], wq_d[:])
            nc.gpsimd.dma_start(wk_sb[:], wk_d[:])
            nc.sync.dma_start(wv_sb[:], wv_d[:])
            wo_sb = w_pool.tile([DC, d], bf16)
            nc.scalar.dma_start(wo_sb[:], wo_d[:])

            # ---- persistent activations ----
            # Q^T and K^T per 512-chunk: [p=head dims, chunk, {q=0,k=1}, 512]
            qk_all = big_pool.tile([DC, NCH, 2, 512], bf16)
            # V natural, augmented with ones columns at 64 (h0) and 129 (h1)
            v_aug = big_pool.tile([128, NT, 130], bf16)
            onescol = const_pool.tile([128, NT], fp32)
            nc.gpsimd.memset(onescol[:], 1.0)
            nc.vector.tensor_copy(v_aug[:, :, HD], onescol[:])
            nc.vector.tensor_copy(v_aug[:, :, 2 * HD + 1], onescol[:])
            ctxr = big_pool.tile([DC, n], bf16)  # normalized ctx^T

            # prefetch the rest of x (after the weight DMAs on each queue)
            for ch in range(1, NCH):
                dq = (nc.sync, nc.gpsimd)[ch % 2]
                dq.dma_start(xchs[ch][:], x2_d[:, ch, :, :])



            # ---- filler thunk machinery ----
            # `filler`: must fully drain within the current chunk (phase1 of
            # the next chunk). `reserve`: out-projections of past chunks,
            # drained only into leftover slots so the ACT-paced late chunks
            # (which have no phase1 left) still get PE filler work.
            filler = []
            reserve = []

            def drain(k):
                for _ in range(min(k, len(filler))):
                    filler.pop(0)()

            def drain_reserve(k):
                for _ in range(min(k, len(reserve))):
                    reserve.pop(0)()

            def phase1_thunks(ch):
                """QKV projection for chunk ch as a list of PE-sized thunks.
                The x DMA is issued immediately (prefetch)."""
                xch = xchs[ch]
                box = {}
                th = []

                def qkmm(which, d0):
                    def t():
                        key = "qk"[which]
                        if key not in box:
                            box[key] = ps.tile(
                                [DC, 512], fp32, tag="w", bufs=2,
                                name=f"{key}_ps",
                            )
                        w_sb = wq_sb if which == 0 else wk_sb
                        for dit in range(d0, d0 + 4):
                            nc.tensor.matmul(
                                box[key][:], w_sb[:, dit, :],
                                xch[:, dit, :],
                                start=(dit == 0), stop=(dit == DIT - 1),
                            )
                    return t

                th += [qkmm(0, 0), qkmm(0, 4)]
                th.append(
                    lambda: nc.vector.tensor_copy(qk_all[:, ch, 0, :], box["q"][:])
                )
                th += [qkmm(1, 0), qkmm(1, 4)]
                th.append(
                    lambda: nc.vector.tensor_copy(qk_all[:, ch, 1, :], box["k"][:])
                )

                def vmm(d0):
                    def t():
                        if "vt" not in box:
                            box["vt"] = ps.tile(
                                [DC, 512], fp32, tag="w", bufs=2, name="vt_ps"
                            )
                        for dit in range(d0, d0 + 4):
                            nc.tensor.matmul(
                                box["vt"][:], wv_sb[:, dit, :], xch[:, dit, :],
                                start=(dit == 0), stop=(dit == DIT - 1),
                            )
                    return t

                th += [vmm(0), vmm(4)]

                def vev():
                    box["vt_t"] = vt_pool.tile(
                        [DC, 512], bf16, tag="vt", bufs=2, name="vt_t"
                    )
                    nc.vector.tensor_copy(box["vt_t"][:], box["vt"][:])

                th.append(vev)

                def vtr(j):
                    def t():
                        ti = ch * 4 + j
                        tp_ps = ps.tile(
                            [128, 128], bf16, tag="w", bufs=2,
                            padded_shape=[128, 1024], name="tp_ps",
                        )
                        nc.tensor.transpose(
                            tp_ps[:], box["vt_t"][:, j * 128 : (j + 1) * 128],
                            ident[:],
                        )
                        nc.vector.tensor_copy(v_aug[:, ti, 0:HD], tp_ps[:, 0:HD])
                        nc.vector.tensor_copy(
                            v_aug[:, ti, HD + 1 : 2 * HD + 1],
                            tp_ps[:, HD : 2 * HD],
                        )
                    return t

                th += [vtr(j) for j in range(4)]
                return th

            def norm_thunks(qc, ctxm):
                """Normalize ctx of chunk qc, fused into its eviction to
                ctxr, as priority thunks. Every reader of the bufs=1 ctx
                slot must be EMITTED before the next chunk's ctxm
                allocation (the caller force-drains these before it)."""
                box = {}

                def t_recip():
                    # split the sums eviction across ACT (idle at chunk
                    # boundaries) and DVE so the serial 1-lane chain that
                    # gates the outer-product matmuls is ~0.5us shorter
                    sums_sb = s_pool.tile(
                        [1, 1024], fp32, tag="sums", bufs=2, name="sums_sb"
                    )
                    nc.scalar.activation(
                        sums_sb[0:1, 0:512], ctxm[HD : HD + 1, 0:512],
                        mybir.ActivationFunctionType.Copy,
                    )
                    nc.vector.tensor_copy(
                        sums_sb[0:1, 512:1024], ctxm[HD : HD + 1, 512:1024]
                    )
                    box["rr"] = s_pool.tile(
                        [1, 1024], fp32, tag="rr", bufs=2, name="rr"
                    )
                    nc.vector.reciprocal_approx_fast(
                        out=box["rr"][:], in_=sums_sb[:]
                    )

                def t_scale(half):
                    def t():
                        if "scale" not in box:
                            box["scale"] = s_pool.tile(
                                [HD, 1024], fp16, tag="scale", bufs=2,
                                name="scale_sb",
                            )
                        sc_ps = ps.tile(
                            [HD, 512], fp32, tag="w", bufs=2, name="sc_ps"
                        )
                        nc.tensor.matmul(
                            sc_ps[:], ones64[:],
                            box["rr"][0:1, half * 512 : half * 512 + 512],
                            start=True, stop=True,
                        )
                        if half == 0:
                            nc.scalar.activation(
                                box["scale"][:, 0:512], sc_ps[:],
                                mybir.ActivationFunctionType.Copy,
                            )
                        else:
                            nc.vector.tensor_copy(
                                box["scale"][:, 512:1024], sc_ps[:]
                            )
                    return t

                qs = slice(qc * 512, qc * 512 + 512)

                def t_ev(h):
                    def t():
                        nc.vector.tensor_mul(
                            ctxr[HD * h : HD * h + HD, qs],
                            ctxm[0:HD, 512 * h : 512 * h + 512],
                            box["scale"][:, 512 * h : 512 * h + 512],
                        )
                    return t

                return [t_recip, t_scale(0), t_scale(1), t_ev(0), t_ev(1)]

            def outproj_thunks(qc, act_evict=False):
                """Out-projection for chunk qc (reads only persistent ctxr),
                safe to defer into the next chunk's attention loop."""
                box = {}
                th = []

                def t_oproj(j, h2):
                    def t():
                        jj = qc * 4 + j
                        gsl = slice(jj * 128, jj * 128 + 128)
                        osl = slice(h2 * 512, (h2 + 1) * 512)
                        if ("o", j) not in box:
                            box[("o", j)] = out_pool.tile(
                                [128, d], bf16, tag="o", bufs=3, name="o_sb"
                            )
                        op_ps = ps.tile(
                            [128, 512], fp32, tag="w", bufs=2, name="op_ps"
                        )
                        nc.tensor.matmul(
                            op_ps[:], ctxr[:, gsl], wo_sb[:, osl],
                            start=True, stop=True,
                        )
                        if act_evict and h2 == 0:
                            nc.scalar.activation(
                                box[("o", j)][:, osl], op_ps[:],
                                mybir.ActivationFunctionType.Copy,
                            )
                        else:
                            nc.vector.tensor_copy(box[("o", j)][:, osl], op_ps[:])
                        if h2 == 1:
                            nc.sync.dma_start(out_d[gsl, :], box[("o", j)][:])
                    return t

                for j in range(4):
                    th += [t_oproj(j, 0), t_oproj(j, 1)]
                return th

            # ---- main loop ----
            for t in phase1_thunks(0):
                t()
            def alloc_ctx():
                return ps.tile(
                    [HD + 1, 1024], fp32, tag="ctx", bufs=1, name="ctxm"
                )

            for ch in range(NCH):
                if ch > 0:
                    reserve.extend(outproj_thunks(ch - 1))
                if ch + 1 < NCH:
                    filler.extend(phase1_thunks(ch + 1))

                qc = ch
                nkt = 4 * (qc + 1)
                ctxm = alloc_ctx()
                pend = []  # deferred PV: (kt, pm, q0), depth 2
                for kt in range(nkt):
                    dd = kt - 4 * qc
                    q0 = 128 * dd if dd > 0 else 0
                    kch, kj = divmod(kt, 4)
                    ks = slice(kj * 128, kj * 128 + 128)
                    # scores^T [kl, h, q]: one PSUM bank per head so the
                    # (0,0)/(64,0) tile_position pair runs concurrently.
                    sm = ps.tile([128, 2, 512], fp32, tag="sm", bufs=2, name="sm")
                    for h in range(2):
                        hs = slice(HD * h, HD * h + HD)
                        nc.tensor.matmul(
                            sm[:, h, q0:512],
                            qk_all[hs, kch, 1, ks],
                            qk_all[hs, qc, 0, q0:512],
                            start=True, stop=True,
                            tile_position=(HD * h, 0),
                        )
                    pm = pm_pool.tile([128, 2, 512], bf16, tag="pm", bufs=4, name="pm")
                    nc.scalar.activation(
                        pm[:, :, q0:512], sm[:, :, q0:512], Exp, scale=SCALE
                    )
                    if dd >= 0:
                        nc.vector.tensor_mul(
                            pm[:, :, q0 : q0 + 128], pm[:, :, q0 : q0 + 128], tri2[:]
                        )
                    # P@V first (its exp has 2 iterations of slack, so it
                    # is nearly always ready), THEN fillers: a filler whose
                    # input is late must not block the ready PV in the
                    # strict-FIFO PE queue
                    if len(pend) >= 2:
                        _emit_pv(nc, ctxm, v_aug, *pend.pop(0), nkt)
                    pend.append((kt, pm, q0))
                    nf = len(filler)
                    if nf:
                        drain(-(-nf // (nkt - kt)))
                    else:
                        drain_reserve(1)
                for p in pend:
                    _emit_pv(nc, ctxm, v_aug, *p, nkt)
                pend.clear()
                for t in norm_thunks(qc, ctxm):
                    t()
                drain(len(filler))

            drain_reserve(len(reserve))
            for t in outproj_thunks(NCH - 1):
                t()

    nc.compile()
    return nc


def _emit_pv(nc, ctxm, v_aug, kt, pm, q0, nkt):
    start = kt == 0
    stop = kt == nkt - 1
    nc.tensor.matmul(
        ctxm[:, q0:512], v_aug[:, kt, 0 : HD + 1], pm[:, 0, q0:512],
        start=start, stop=stop,
    )
    nc.tensor.matmul(
        ctxm[:, 512 + q0 : 1024], v_aug[:, kt, HD + 1 : 2 * HD + 2],
        pm[:, 1, q0:512],
        start=start, stop=stop,
    )


_NC_CACHE = {}


def _get_nc(n=SEQ):
    if n not in _NC_CACHE:
        _NC_CACHE[n] = build_bass(n)
    return _NC_CACHE[n]


def make_in_maps(x, W_q, W_k, W_v, W_o):
    import ml_dtypes

    bf16 = ml_dtypes.bfloat16
    n = x.shape[-2]
    xT = np.asarray(x, dtype=np.float32).reshape(n, D).T  # [d, n]
    # x2[p, ch, dit, c] = xT[dit*128 + p, ch*512 + c]
    x2 = np.ascontiguousarray(
        xT.reshape(D // 128, 128, n // 512, 512).transpose(1, 2, 0, 3)
    ).astype(bf16)

    def wperm(W, s):
        # w2[p, dit, c] = W[dit*128 + p, s][c]
        Ws = np.asarray(W, np.float32)[:, s]
        return np.ascontiguousarray(
            Ws.reshape(D // 128, 128, DC).transpose(1, 0, 2)
        ).astype(bf16)

    in_maps = []
    for c in range(N_CORES):
        s = slice(c * DC, (c + 1) * DC)
        in_maps.append(
            {
                "x2": x2,
                "wq": wperm(W_q, s),
                "wk": wperm(W_k, s),
                "wv": wperm(W_v, s),
                "wo": np.ascontiguousarray(np.asarray(W_o, np.float32)[s, :]).astype(bf16),
            }
        )
    return in_maps


def kernel(x, W_q, W_k, W_v, W_o, b_o):
    from concourse import bass_utils

    x = np.asarray(x)
    b, n, _ = x.shape
    assert b == 1 and n == SEQ

    nc = _get_nc(n)
    in_maps = make_in_maps(x, W_q, W_k, W_v, W_o)
    res = bass_utils.run_bass_kernel_spmd(nc, in_maps, list(range(N_CORES)))
    acc = np.zeros((n, D), dtype=np.float64)
    for r in res.results:
        acc += r["out"].astype(np.float64)
    acc += np.asarray(b_o, np.float64)[None, :]
    return acc.astype(np.float32).reshape(1, n, D)


# revision 54
# speedup vs baseline: 1.0027x; 1.0027x over previous
"""Trainium2 Bass kernel for causal multi-head attention.

Problem: x[1,4096,1024] -> MHA(16 heads, head_dim 64, causal) -> out[1,4096,1024]
  q,k,v = x @ W_{q,k,v}; scores = q k^T / 8 (causal); out = softmax(scores) v @ W_o + b_o

Sharding: tensor-parallel over heads, 2 heads (128 feature dims) per core.
Each core computes a full-width partial output ctx_c @ W_o[slice_c] which the
host sums over the 8 cores (row-parallel out-projection).

Design:
- The attention inner loop (scores -> exp -> P@V) is ACT-paced; other
  engines' work is software-pipelined INTO it as "filler" thunks, because
  engine queues are strict FIFO: if PE's next instruction depends on a
  pending exp, PE idles and the HAM clock gate drops it to 1.2 GHz.
  Fillers = next chunk's QKV-projection matmuls (phase1) + previous
  chunks' out-projections (kept in a carry-over reserve so the ACT-heavy
  late chunks still get PE work). P@V for k-tile kt is emitted two
  iterations behind scores(kt) so exp(kt) latency is hidden.
- A dependency-free burst of warmup matmuls fills the startup DMA wait so
  the PE enters the pipeline at 2.4 GHz (the schedule is otherwise
  bistable between a ~255us warm mode and a ~305us cold-oscillating one).
- Scores for both heads of one 128-k-tile go in one [128, 2(h), 512] fp32
  PSUM tile (one bank per head; the tile_position pair runs concurrently);
  one ACTIVATE(exp) covers both heads and applies the 1/8 softmax scale
  via its free affine input.
- Diagonal score tiles are column-restricted (fully-masked columns never
  touch any engine); the in-block causal triangle is zeroed by one
  [128, 2, 128] bf16 DVE multiply.
- ctx is normalized BEFORE the out-projection: per-q reciprocals of the
  softmax sums (picked up by an appended ones-column in P@V) via
  reciprocal_approx_fast, broadcast to a [64, 512] matrix by a PE
  outer-product, fused into the DVE eviction of ctx^T. The out-projection
  is then one contraction-128 matmul per output block with no DVE math.
- x and the QKV weights are host-pre-permuted so every DMA line is
  contiguous per partition; the whole x is prefetched up front across the
  sync/gpsimd queues. Output partials are bf16; the host sums in fp64.

kernel(**inputs) takes the FULL unsharded inputs and returns the FULL output.
"""

import sys

import numpy as np

for _p in ("/opt/trn_rl_repo", "/root/.axon_site/_ro/trn_rl_repo"):
    if _p not in sys.path:
        try:
            import concourse  # noqa: F401

            break
        except ImportError:
            sys.path.insert(0, _p)

N_CORES = 8
SEQ = 4096
D = 1024
DC = 128  # per-core slice of the head dim (2 heads x 64)
HD = 64


def build_bass(n=SEQ, d=D):
    """Trace the per-core SPMD Bass program. n = sequence length."""
    import concourse.bacc as bacc
    import concourse.mybir as mybir
    import concourse.tile as tile
    from concourse.masks import make_identity

    fp32 = mybir.dt.float32
    fp16 = mybir.dt.float16
    bf16 = mybir.dt.bfloat16
    Exp = mybir.ActivationFunctionType.Exp

    assert n % 512 == 0 and d % 128 == 0
    NT = n // 128  # 128-row seq tiles
    NCH = n // 512  # 512-col seq chunks
    DIT = d // 128  # input-dim 128-tiles
    SCALE = 1.0 / float(np.sqrt(HD))

    nc = bacc.Bacc("TRN2", target_bir_lowering=False)

    # host-pre-permuted layouts so every DMA line is contiguous per
    # partition: x2[p, ch, dit, c], w*2[p, dit, c]
    x2_d = nc.dram_tensor("x2", (128, NCH, DIT, 512), bf16, kind="ExternalInput")
    wq_d = nc.dram_tensor("wq", (128, DIT, DC), bf16, kind="ExternalInput")
    wk_d = nc.dram_tensor("wk", (128, DIT, DC), bf16, kind="ExternalInput")
    wv_d = nc.dram_tensor("wv", (128, DIT, DC), bf16, kind="ExternalInput")
    wo_d = nc.dram_tensor("wo", (DC, d), bf16, kind="ExternalInput")
    out_d = nc.dram_tensor("out", (n, d), bf16, kind="ExternalOutput")

    with tile.TileContext(nc) as tc:
        with (
            tc.tile_pool(name="const", bufs=1) as const_pool,
            tc.tile_pool(name="weights", bufs=1) as w_pool,
            tc.tile_pool(name="big", bufs=1) as big_pool,
            tc.tile_pool(name="xin", bufs=2) as xin_pool,
            tc.tile_pool(name="vt", bufs=2) as vt_pool,
            tc.tile_pool(name="pm", bufs=4) as pm_pool,
            tc.tile_pool(name="small", bufs=2) as s_pool,
            tc.tile_pool(name="outsb", bufs=3) as out_pool,
            tc.tile_pool(name="ps", bufs=1, space="PSUM") as ps,
        ):
            # Dependency-free warmup matmuls, first thing: back-to-back PE
            # work so the HAM clock gate reaches 8/8 before the pipeline
            # starts, covering the startup DMA wait (the schedule is
            # bistable: a ~2.7us PE idle seam between warmup end and the
            # wq/xch0 DMA landing sends the whole run into a cold
            # ~305us mode). DVE memset so warmup isn't queued behind the
            # gpsimd constant setup.
            scr_sb = const_pool.tile([128, 512], bf16)
            nc.vector.memset(scr_sb[:], 0.0)
            for _ in range(28):
                scr_ps = ps.tile(
                    [128, 512], fp32, tag="w", bufs=2, name="scr_ps"
                )
                nc.tensor.matmul(
                    scr_ps[:], scr_sb[:, 0:128], scr_sb[:], start=True, stop=True
                )

            # ---- x chunk 0 first on the sync queue so it lands before
            # the warmup ends (it gates the first real matmul) ----
            xchs = []
            for ch in range(NCH):
                xchs.append(
                    xin_pool.tile(
                        [128, DIT, 512], bf16, tag=f"xch{ch}", bufs=1, name="xch"
                    )
                )
            nc.sync.dma_start(xchs[0][:], x2_d[:, 0, :, :])

            # ---- constants ----
            ident = const_pool.tile([128, 128], bf16)
            make_identity(nc, ident[:])
            # tri2[kl, h, q'] = 1 if q' >= kl else 0 -- the within-block
            # causal triangle, replicated over the two heads.
            tri2 = const_pool.tile([128, 2, 128], bf16)
            nc.gpsimd.memset(tri2[:], 1.0)
            for h in range(2):
                nc.gpsimd.affine_select(
                    out=tri2[:, h, :],
                    in_=tri2[:, h, :],
                    compare_op=mybir.AluOpType.is_ge,
                    fill=0.0,
                    base=0,
                    pattern=[[1, 128]],
                    channel_multiplier=-1,
                )
            ones64 = const_pool.tile([1, HD], fp32)
            nc.gpsimd.memset(ones64[:], 1.0)

            # ---- weights ----
            wq_sb = w_pool.tile([128, DIT, DC], bf16)
            wk_sb = w_pool.tile([128, DIT, DC], bf16)
            wv_sb = w_pool.tile([128, DIT, DC], bf16)
            nc.scalar.dma_start(wq_sb[:], wq_d[:])
            nc.gpsimd.dma_start(wk_sb[:], wk_d[:])
            nc.sync.dma_start(wv_sb[:], wv_d[:])
            wo_sb = w_pool.tile([DC, d], bf16)
            nc.scalar.dma_start(wo_sb[:], wo_d[:])

            # ---- persistent activations ----
            # Q^T and K^T per 512-chunk: [p=head dims, chunk, {q=0,k=1}, 512]
            qk_all = big_pool.tile([DC, NCH, 2, 512], bf16)
            # V natural, augmented with ones columns at 64 (h0) and 129 (h1)
            v_aug = big_pool.tile([128, NT, 130], bf16)
            onescol = const_pool.tile([128, NT], fp32)
            nc.gpsimd.memset(onescol[:], 1.0)
            nc.vector.tensor_copy(v_aug[:, :, HD], onescol[:])
            nc.vector.tensor_copy(v_aug[:, :, 2 * HD + 1], onescol[:])
            ctxr = big_pool.tile([DC, n], bf16)  # normalized ctx^T

            # prefetch the rest of x (after the weight DMAs on each queue)
            for ch in range(1, NCH):
                dq = (nc.sync, nc.gpsimd)[ch % 2]
                dq.dma_start(xchs[ch][:], x2_d[:, ch, :, :])



            # ---- filler thunk machinery ----
            # `filler`: must fully drain within the current chunk (phase1 of
            # the next chunk). `reserve`: out-projections of past chunks,
            # drained only into leftover slots so the ACT-paced late chunks
            # (which have no phase1 left) still get PE filler work.
            filler = []
            reserve = []

            def drain(k):
                for _ in range(min(k, len(filler))):
                    filler.pop(0)()

            def drain_reserve(k):
                for _ in range(min(k, len(reserve))):
                    reserve.pop(0)()

            def phase1_thunks(ch):
                """QKV projection for chunk ch as a list of PE-sized thunks.
                The x DMA is issued immediately (prefetch)."""
                xch = xchs[ch]
                box = {}
                th = []

                def qkmm(which, d0):
                    def t():
                        key = "qk"[which]
                        if key not in box:
                            box[key] = ps.tile(
                                [DC, 512], fp32, tag="w", bufs=2,
                                name=f"{key}_ps",
                            )
                        w_sb = wq_sb if which == 0 else wk_sb
                        for dit in range(d0, d0 + 4):
                            nc.tensor.matmul(
                                box[key][:], w_sb[:, dit, :],
                                xch[:, dit, :],
                                start=(dit == 0), stop=(dit == DIT - 1),
                            )
                    return t

                th += [qkmm(0, 0), qkmm(0, 4)]
                th.append(
                    lambda: nc.vector.tensor_copy(qk_all[:, ch, 0, :], box["q"][:])
                )
                th += [qkmm(1, 0), qkmm(1, 4)]
                th.append(
                    lambda: nc.vector.tensor_copy(qk_all[:, ch, 1, :], box["k"][:])
                )

                def vmm(d0):
                    def t():
                        if "vt" not in box:
                            box["vt"] = ps.tile(
                                [DC, 512], fp32, tag="w", bufs=2, name="vt_ps"
                            )
                        for dit in range(d0, d0 + 4):
                            nc.tensor.matmul(
                                box["vt"][:], wv_sb[:, dit, :], xch[:, dit, :],
                                start=(dit == 0), stop=(dit == DIT - 1),
                            )
                    return t

                th += [vmm(0), vmm(4)]

                def vev():
                    box["vt_t"] = vt_pool.tile(
                        [DC, 512], bf16, tag="vt", bufs=2, name="vt_t"
                    )
                    nc.vector.tensor_copy(box["vt_t"][:], box["vt"][:])

                th.append(vev)

                def vtr(j):
                    def t():
                        ti = ch * 4 + j
                        tp_ps = ps.tile(
                            [128, 128], bf16, tag="w", bufs=2,
                            padded_shape=[128, 1024], name="tp_ps",
                        )
                        nc.tensor.transpose(
                            tp_ps[:], box["vt_t"][:, j * 128 : (j + 1) * 128],
                            ident[:],
                        )
                        nc.vector.tensor_copy(v_aug[:, ti, 0:HD], tp_ps[:, 0:HD])
                        nc.vector.tensor_copy(
                            v_aug[:, ti, HD + 1 : 2 * HD + 1],
                            tp_ps[:, HD : 2 * HD],
                        )
                    return t

                th += [vtr(j) for j in range(4)]
                return th

            def norm_thunks(qc, ctxm):
                """Normalize ctx of chunk qc, fused into its eviction to
                ctxr, as priority thunks. Every reader of the bufs=1 ctx
                slot must be EMITTED before the next chunk's ctxm
                allocation (the caller force-drains these before it)."""
                box = {}

                def t_recip():
                    sums_sb = s_pool.tile(
                        [1, 1024], fp32, tag="sums", bufs=2, name="sums_sb"
                    )
                    nc.vector.tensor_copy(sums_sb[:], ctxm[HD : HD + 1, :])
                    box["rr"] = s_pool.tile(
                        [1, 1024], fp32, tag="rr", bufs=2, name="rr"
                    )
                    nc.vector.reciprocal_approx_fast(
                        out=box["rr"][:], in_=sums_sb[:]
                    )

                def t_scale(half):
                    def t():
                        if "scale" not in box:
                            box["scale"] = s_pool.tile(
                                [HD, 1024], fp16, tag="scale", bufs=2,
                                name="scale_sb",
                            )
                        sc_ps = ps.tile(
                            [HD, 512], fp32, tag="w", bufs=2, name="sc_ps"
                        )
                        nc.tensor.matmul(
                            sc_ps[:], ones64[:],
                            box["rr"][0:1, half * 512 : half * 512 + 512],
                            start=True, stop=True,
                        )
                        nc.vector.tensor_copy(
                            box["scale"][:, half * 512 : half * 512 + 512],
                            sc_ps[:],
                        )
                    return t

                qs = slice(qc * 512, qc * 512 + 512)

                def t_ev(h):
                    def t():
                        nc.vector.tensor_mul(
                            ctxr[HD * h : HD * h + HD, qs],
                            ctxm[0:HD, 512 * h : 512 * h + 512],
                            box["scale"][:, 512 * h : 512 * h + 512],
                        )
                    return t

                return [t_recip, t_scale(0), t_scale(1), t_ev(0), t_ev(1)]

            def outproj_thunks(qc, act_evict=False):
                """Out-projection for chunk qc (reads only persistent ctxr),
                safe to defer into the next chunk's attention loop."""
                box = {}
                th = []

                def t_oproj(j, h2):
                    def t():
                        jj = qc * 4 + j
                        gsl = slice(jj * 128, jj * 128 + 128)
                        osl = slice(h2 * 512, (h2 + 1) * 512)
                        if ("o", j) not in box:
                            box[("o", j)] = out_pool.tile(
                                [128, d], bf16, tag="o", bufs=3, name="o_sb"
                            )
                        op_ps = ps.tile(
                            [128, 512], fp32, tag="w", bufs=2, name="op_ps"
                        )
                        nc.tensor.matmul(
                            op_ps[:], ctxr[:, gsl], wo_sb[:, osl],
                            start=True, stop=True,
                        )
                        if act_evict and h2 == 0:
                            nc.scalar.activation(
                                box[("o", j)][:, osl], op_ps[:],
                                mybir.ActivationFunctionType.Copy,
                            )
                        else:
                            nc.vector.tensor_copy(box[("o", j)][:, osl], op_ps[:])
                        if h2 == 1:
                            nc.sync.dma_start(out_d[gsl, :], box[("o", j)][:])
                    return t

                for j in range(4):
                    th += [t_oproj(j, 0), t_oproj(j, 1)]
                return th

            def emit_sc(qc2, kt):
                """Scores^T + exp (+ triangle mask) for (q-chunk qc2, k-tile
                kt); one PSUM bank per head so the (0,0)/(64,0)
                tile_position pair runs concurrently."""
                dd = kt - 4 * qc2
                q0 = 128 * dd if dd > 0 else 0
                kch, kj = divmod(kt, 4)
                ks = slice(kj * 128, kj * 128 + 128)
                sm = ps.tile([128, 2, 512], fp32, tag="sm", bufs=2, name="sm")
                for h in range(2):
                    hs = slice(HD * h, HD * h + HD)
                    nc.tensor.matmul(
                        sm[:, h, q0:512],
                        qk_all[hs, kch, 1, ks],
                        qk_all[hs, qc2, 0, q0:512],
                        start=True, stop=True,
                        tile_position=(HD * h, 0),
                    )
                pm = pm_pool.tile([128, 2, 512], bf16, tag="pm", bufs=4, name="pm")
                nc.scalar.activation(
                    pm[:, :, q0:512], sm[:, :, q0:512], Exp, scale=SCALE
                )
                if dd >= 0:
                    nc.vector.tensor_mul(
                        pm[:, :, q0 : q0 + 128], pm[:, :, q0 : q0 + 128], tri2[:]
                    )
                return (kt, pm, q0)

            # ---- main loop ----
            pre = []
            for t in phase1_thunks(0):
                t()
            def alloc_ctx():
                return ps.tile(
                    [HD + 1, 1024], fp32, tag="ctx", bufs=1, name="ctxm"
                )

            for ch in range(NCH):
                if ch > 0:
                    reserve.extend(outproj_thunks(ch - 1))
                if ch + 1 < NCH:
                    filler.extend(phase1_thunks(ch + 1))

                qc = ch
                nkt = 4 * (qc + 1)
                ctxm = alloc_ctx()
                pend = pre  # deferred PV (kt, pm, q0): pre-seeded by hoist
                pre = []
                for kt in range(len(pend), nkt):
                    tup = emit_sc(qc, kt)
                    # P@V first (its exp has 2 iterations of slack, so it
                    # is nearly always ready), THEN fillers: a filler whose
                    # input is late must not block the ready PV in the
                    # strict-FIFO PE queue
                    if len(pend) >= 2:
                        _emit_pv(nc, ctxm, v_aug, *pend.pop(0), nkt)
                    pend.append(tup)
                    nf = len(filler)
                    if nf:
                        drain(-(-nf // (nkt - kt)))
                    else:
                        drain_reserve(1)
                for p in pend:
                    _emit_pv(nc, ctxm, v_aug, *p, nkt)
                pend.clear()
                drain(len(filler))
                if ch + 1 < NCH:
                    # hoist the next chunk's first two score tiles ahead of
                    # the norm chain: real PE work covering the reciprocal
                    # wait that gates the outer-product matmuls
                    pre = [emit_sc(ch + 1, 0), emit_sc(ch + 1, 1)]
                for t in norm_thunks(qc, ctxm):
                    t()

            drain_reserve(len(reserve))
            for t in outproj_thunks(NCH - 1):
                t()

    nc.compile()
    return nc


def _emit_pv(nc, ctxm, v_aug, kt, pm, q0, nkt):
    start = kt == 0
    stop = kt == nkt - 1
    nc.tensor.matmul(
        ctxm[:, q0:512], v_aug[:, kt, 0 : HD + 1], pm[:, 0, q0:512],
        start=start, stop=stop,
    )
    nc.tensor.matmul(
        ctxm[:, 512 + q0 : 1024], v_aug[:, kt, HD + 1 : 2 * HD + 2],
        pm[:, 1, q0:512],
        start=start, stop=stop,
    )


_NC_CACHE = {}


def _get_nc(n=SEQ):
    if n not in _NC_CACHE:
        _NC_CACHE[n] = build_bass(n)
    return _NC_CACHE[n]


def make_in_maps(x, W_q, W_k, W_v, W_o):
    import ml_dtypes

    bf16 = ml_dtypes.bfloat16
    n = x.shape[-2]
    xT = np.asarray(x, dtype=np.float32).reshape(n, D).T  # [d, n]
    # x2[p, ch, dit, c] = xT[dit*128 + p, ch*512 + c]
    x2 = np.ascontiguousarray(
        xT.reshape(D // 128, 128, n // 512, 512).transpose(1, 2, 0, 3)
    ).astype(bf16)

    def wperm(W, s):
        # w2[p, dit, c] = W[dit*128 + p, s][c]
        Ws = np.asarray(W, np.float32)[:, s]
        return np.ascontiguousarray(
            Ws.reshape(D // 128, 128, DC).transpose(1, 0, 2)
        ).astype(bf16)

    in_maps = []
    for c in range(N_CORES):
        s = slice(c * DC, (c + 1) * DC)
        in_maps.append(
            {
                "x2": x2,
                "wq": wperm(W_q, s),
                "wk": wperm(W_k, s),
                "wv": wperm(W_v, s),
                "wo": np.ascontiguousarray(np.asarray(W_o, np.float32)[s, :]).astype(bf16),
            }
        )
    return in_maps


def kernel(x, W_q, W_k, W_v, W_o, b_o):
    from concourse import bass_utils

    x = np.asarray(x)
    b, n, _ = x.shape
    assert b == 1 and n == SEQ

    nc = _get_nc(n)
    in_maps = make_in_maps(x, W_q, W_k, W_v, W_o)
    res = bass_utils.run_bass_kernel_spmd(nc, in_maps, list(range(N_CORES)))
    acc = np.zeros((n, D), dtype=np.float64)
    for r in res.results:
        acc += r["out"].astype(np.float64)
    acc += np.asarray(b_o, np.float64)[None, :]
    return acc.astype(np.float32).reshape(1, n, D)


# revision 55
# speedup vs baseline: 1.0351x; 1.0323x over previous
"""Trainium2 Bass kernel for causal multi-head attention.

Problem: x[1,4096,1024] -> MHA(16 heads, head_dim 64, causal) -> out[1,4096,1024]
  q,k,v = x @ W_{q,k,v}; scores = q k^T / 8 (causal); out = softmax(scores) v @ W_o + b_o

Sharding: tensor-parallel over heads, 2 heads (128 feature dims) per core.
Each core computes a full-width partial output ctx_c @ W_o[slice_c] which the
host sums over the 8 cores (row-parallel out-projection).

Design:
- The attention inner loop (scores -> exp -> P@V) is ACT-paced; other
  engines' work is software-pipelined INTO it as "filler" thunks, because
  engine queues are strict FIFO: if PE's next instruction depends on a
  pending exp, PE idles and the HAM clock gate drops it to 1.2 GHz.
  Fillers = next chunk's QKV-projection matmuls (phase1) + previous
  chunks' out-projections (kept in a carry-over reserve so the ACT-heavy
  late chunks still get PE work). P@V for k-tile kt is emitted two
  iterations behind scores(kt) so exp(kt) latency is hidden.
- A dependency-free burst of warmup matmuls fills the startup DMA wait so
  the PE enters the pipeline at 2.4 GHz (the schedule is otherwise
  bistable between a ~255us warm mode and a ~305us cold-oscillating one).
- Scores for both heads of one 128-k-tile go in one [128, 2(h), 512] fp32
  PSUM tile (one bank per head; the tile_position pair runs concurrently);
  one ACTIVATE(exp) covers both heads and applies the 1/8 softmax scale
  via its free affine input.
- Diagonal score tiles are column-restricted (fully-masked columns never
  touch any engine); the in-block causal triangle is zeroed by one
  [128, 2, 128] bf16 DVE multiply.
- ctx is normalized BEFORE the out-projection: per-q reciprocals of the
  softmax sums (picked up by an appended ones-column in P@V) via
  reciprocal_approx_fast, broadcast to a [64, 512] matrix by a PE
  outer-product, fused into the DVE eviction of ctx^T. The out-projection
  is then one contraction-128 matmul per output block with no DVE math.
- x and the QKV weights are host-pre-permuted so every DMA line is
  contiguous per partition; the whole x is prefetched up front across the
  sync/gpsimd queues. Output partials are bf16; the host sums in fp64.

kernel(**inputs) takes the FULL unsharded inputs and returns the FULL output.
"""

import sys

import numpy as np

for _p in ("/opt/trn_rl_repo", "/root/.axon_site/_ro/trn_rl_repo"):
    if _p not in sys.path:
        try:
            import concourse  # noqa: F401

            break
        except ImportError:
            sys.path.insert(0, _p)

N_CORES = 8
SEQ = 4096
D = 1024
DC = 128  # per-core slice of the head dim (2 heads x 64)
HD = 64


def build_bass(n=SEQ, d=D):
    """Trace the per-core SPMD Bass program. n = sequence length."""
    import concourse.bacc as bacc
    import concourse.mybir as mybir
    import concourse.tile as tile
    from concourse.masks import make_identity

    fp32 = mybir.dt.float32
    fp16 = mybir.dt.float16
    bf16 = mybir.dt.bfloat16
    Exp = mybir.ActivationFunctionType.Exp

    assert n % 512 == 0 and d % 128 == 0
    NT = n // 128  # 128-row seq tiles
    NCH = n // 512  # 512-col seq chunks
    DIT = d // 128  # input-dim 128-tiles
    SCALE = 1.0 / float(np.sqrt(HD))

    nc = bacc.Bacc("TRN2", target_bir_lowering=False)

    # host-pre-permuted layouts so every DMA line is contiguous per
    # partition: x2[p, ch, dit, c], w*2[p, dit, c]
    x2_d = nc.dram_tensor("x2", (128, NCH, DIT, 512), bf16, kind="ExternalInput")
    wq_d = nc.dram_tensor("wq", (128, DIT, DC), bf16, kind="ExternalInput")
    wk_d = nc.dram_tensor("wk", (128, DIT, DC), bf16, kind="ExternalInput")
    wv_d = nc.dram_tensor("wv", (128, DIT, DC), bf16, kind="ExternalInput")
    wo_d = nc.dram_tensor("wo", (DC, d), bf16, kind="ExternalInput")
    out_d = nc.dram_tensor("out", (n, d), bf16, kind="ExternalOutput")

    with tile.TileContext(nc) as tc:
        with (
            tc.tile_pool(name="const", bufs=1) as const_pool,
            tc.tile_pool(name="weights", bufs=1) as w_pool,
            tc.tile_pool(name="big", bufs=1) as big_pool,
            tc.tile_pool(name="xin", bufs=2) as xin_pool,
            tc.tile_pool(name="vt", bufs=2) as vt_pool,
            tc.tile_pool(name="pm", bufs=4) as pm_pool,
            tc.tile_pool(name="small", bufs=2) as s_pool,
            tc.tile_pool(name="outsb", bufs=3) as out_pool,
            tc.tile_pool(name="ps", bufs=1, space="PSUM") as ps,
        ):
            # Dependency-free warmup matmuls, first thing: back-to-back PE
            # work so the HAM clock gate reaches 8/8 before the pipeline
            # starts, covering the startup DMA wait (the schedule is
            # bistable: a ~2.7us PE idle seam between warmup end and the
            # wq/xch0 DMA landing sends the whole run into a cold
            # ~305us mode). DVE memset so warmup isn't queued behind the
            # gpsimd constant setup.
            scr_sb = const_pool.tile([128, 512], bf16)
            nc.vector.memset(scr_sb[:], 0.0)
            for _ in range(28):
                scr_ps = ps.tile(
                    [128, 512], fp32, tag="w", bufs=2, name="scr_ps"
                )
                nc.tensor.matmul(
                    scr_ps[:], scr_sb[:, 0:128], scr_sb[:], start=True, stop=True
                )

            # ---- x chunk 0 first on the sync queue so it lands before
            # the warmup ends (it gates the first real matmul) ----
            xchs = []
            for ch in range(NCH):
                xchs.append(
                    xin_pool.tile(
                        [128, DIT, 512], bf16, tag=f"xch{ch}", bufs=1, name="xch"
                    )
                )
            nc.sync.dma_start(xchs[0][:], x2_d[:, 0, :, :])

            # ---- constants ----
            ident = const_pool.tile([128, 128], bf16)
            make_identity(nc, ident[:])
            # tri2[kl, h, q'] = 1 if q' >= kl else 0 -- the within-block
            # causal triangle, replicated over the two heads.
            tri2 = const_pool.tile([128, 2, 128], bf16)
            nc.gpsimd.memset(tri2[:], 1.0)
            for h in range(2):
                nc.gpsimd.affine_select(
                    out=tri2[:, h, :],
                    in_=tri2[:, h, :],
                    compare_op=mybir.AluOpType.is_ge,
                    fill=0.0,
                    base=0,
                    pattern=[[1, 128]],
                    channel_multiplier=-1,
                )
            ones64 = const_pool.tile([1, HD], fp32)
            nc.gpsimd.memset(ones64[:], 1.0)

            # ---- weights ----
            wq_sb = w_pool.tile([128, DIT, DC], bf16)
            wk_sb = w_pool.tile([128, DIT, DC], bf16)
            wv_sb = w_pool.tile([128, DIT, DC], bf16)
            nc.scalar.dma_start(wq_sb[:], wq_d[:])
            nc.gpsimd.dma_start(wk_sb[:], wk_d[:])
            nc.sync.dma_start(wv_sb[:], wv_d[:])
            wo_sb = w_pool.tile([DC, d], bf16)
            nc.scalar.dma_start(wo_sb[:], wo_d[:])

            # ---- persistent activations ----
            # Q^T and K^T per 512-chunk: [p=head dims, chunk, {q=0,k=1}, 512]
            qk_all = big_pool.tile([DC, NCH, 2, 512], bf16)
            # V natural, augmented with ones columns at 64 (h0) and 129 (h1)
            v_aug = big_pool.tile([128, NT, 130], bf16)
            onescol = const_pool.tile([128, NT], fp32)
            nc.gpsimd.memset(onescol[:], 1.0)
            nc.vector.tensor_copy(v_aug[:, :, HD], onescol[:])
            nc.vector.tensor_copy(v_aug[:, :, 2 * HD + 1], onescol[:])
            ctxr = big_pool.tile([DC, n], bf16)  # normalized ctx^T

            # prefetch the rest of x (after the weight DMAs on each queue)
            for ch in range(1, NCH):
                dq = (nc.sync, nc.gpsimd)[ch % 2]
                dq.dma_start(xchs[ch][:], x2_d[:, ch, :, :])



            # ---- filler thunk machinery ----
            # `filler`: must fully drain within the current chunk (phase1 of
            # the next chunk). `reserve`: out-projections of past chunks,
            # drained only into leftover slots so the ACT-paced late chunks
            # (which have no phase1 left) still get PE filler work.
            filler = []
            reserve = []

            def drain(k):
                for _ in range(min(k, len(filler))):
                    filler.pop(0)()

            def drain_reserve(k):
                for _ in range(min(k, len(reserve))):
                    reserve.pop(0)()

            def phase1_thunks(ch):
                """QKV projection for chunk ch as a list of PE-sized thunks.
                The x DMA is issued immediately (prefetch)."""
                xch = xchs[ch]
                box = {}
                th = []

                def qkmm(which, d0):
                    def t():
                        key = "qk"[which]
                        if key not in box:
                            box[key] = ps.tile(
                                [DC, 512], fp32, tag="w", bufs=2,
                                name=f"{key}_ps",
                            )
                        w_sb = wq_sb if which == 0 else wk_sb
                        for dit in range(d0, d0 + 4):
                            nc.tensor.matmul(
                                box[key][:], w_sb[:, dit, :],
                                xch[:, dit, :],
                                start=(dit == 0), stop=(dit == DIT - 1),
                            )
                    return t

                th += [qkmm(0, 0), qkmm(0, 4)]
                th.append(
                    lambda: nc.vector.tensor_copy(qk_all[:, ch, 0, :], box["q"][:])
                )
                th += [qkmm(1, 0), qkmm(1, 4)]
                th.append(
                    lambda: nc.vector.tensor_copy(qk_all[:, ch, 1, :], box["k"][:])
                )

                def vmm(d0):
                    def t():
                        if "vt" not in box:
                            box["vt"] = ps.tile(
                                [DC, 512], fp32, tag="w", bufs=2, name="vt_ps"
                            )
                        for dit in range(d0, d0 + 4):
                            nc.tensor.matmul(
                                box["vt"][:], wv_sb[:, dit, :], xch[:, dit, :],
                                start=(dit == 0), stop=(dit == DIT - 1),
                            )
                    return t

                th += [vmm(0), vmm(4)]

                def vev():
                    box["vt_t"] = vt_pool.tile(
                        [DC, 512], bf16, tag="vt", bufs=2, name="vt_t"
                    )
                    nc.vector.tensor_copy(box["vt_t"][:], box["vt"][:])

                th.append(vev)

                def vtr(j):
                    def t():
                        ti = ch * 4 + j
                        tp_ps = ps.tile(
                            [128, 128], bf16, tag="w", bufs=2,
                            padded_shape=[128, 1024], name="tp_ps",
                        )
                        nc.tensor.transpose(
                            tp_ps[:], box["vt_t"][:, j * 128 : (j + 1) * 128],
                            ident[:],
                        )
                        nc.vector.tensor_copy(v_aug[:, ti, 0:HD], tp_ps[:, 0:HD])
                        nc.vector.tensor_copy(
                            v_aug[:, ti, HD + 1 : 2 * HD + 1],
                            tp_ps[:, HD : 2 * HD],
                        )
                    return t

                th += [vtr(j) for j in range(4)]
                return th

            def norm_thunks(qc, ctxm):
                """Normalize ctx of chunk qc, fused into its eviction to
                ctxr, as priority thunks. Every reader of the bufs=1 ctx
                slot must be EMITTED before the next chunk's ctxm
                allocation (the caller force-drains these before it)."""
                box = {}

                def t_recip():
                    sums_sb = s_pool.tile(
                        [1, 1024], fp32, tag="sums", bufs=2, name="sums_sb"
                    )
                    nc.vector.tensor_copy(sums_sb[:], ctxm[HD : HD + 1, :])
                    box["rr"] = s_pool.tile(
                        [1, 1024], fp32, tag="rr", bufs=2, name="rr"
                    )
                    nc.vector.reciprocal_approx_fast(
                        out=box["rr"][:], in_=sums_sb[:]
                    )

                def t_scale(half):
                    def t():
                        if "scale" not in box:
                            box["scale"] = s_pool.tile(
                                [HD, 1024], fp16, tag="scale", bufs=2,
                                name="scale_sb",
                            )
                        sc_ps = ps.tile(
                            [HD, 512], fp32, tag="w", bufs=2, name="sc_ps"
                        )
                        nc.tensor.matmul(
                            sc_ps[:], ones64[:],
                            box["rr"][0:1, half * 512 : half * 512 + 512],
                            start=True, stop=True,
                        )
                        nc.vector.tensor_copy(
                            box["scale"][:, half * 512 : half * 512 + 512],
                            sc_ps[:],
                        )
                    return t

                qs = slice(qc * 512, qc * 512 + 512)

                def t_ev(h):
                    def t():
                        nc.vector.tensor_mul(
                            ctxr[HD * h : HD * h + HD, qs],
                            ctxm[0:HD, 512 * h : 512 * h + 512],
                            box["scale"][:, 512 * h : 512 * h + 512],
                        )
                    return t

                return [t_recip, t_scale(0), t_scale(1), t_ev(0), t_ev(1)]

            def outproj_thunks(qc, act_evict=False):
                """Out-projection for chunk qc (reads only persistent ctxr),
                safe to defer into the next chunk's attention loop."""
                box = {}
                th = []

                def t_oproj(j, h2):
                    def t():
                        jj = qc * 4 + j
                        gsl = slice(jj * 128, jj * 128 + 128)
                        osl = slice(h2 * 512, (h2 + 1) * 512)
                        if ("o", j) not in box:
                            box[("o", j)] = out_pool.tile(
                                [128, d], bf16, tag="o", bufs=3, name="o_sb"
                            )
                        op_ps = ps.tile(
                            [128, 512], fp32, tag="w", bufs=2, name="op_ps"
                        )
                        nc.tensor.matmul(
                            op_ps[:], ctxr[:, gsl], wo_sb[:, osl],
                            start=True, stop=True,
                        )
                        if act_evict and h2 == 0:
                            nc.scalar.activation(
                                box[("o", j)][:, osl], op_ps[:],
                                mybir.ActivationFunctionType.Copy,
                            )
                        else:
                            nc.vector.tensor_copy(box[("o", j)][:, osl], op_ps[:])
                        if h2 == 1:
                            nc.sync.dma_start(out_d[gsl, :], box[("o", j)][:])
                    return t

                for j in range(4):
                    th += [t_oproj(j, 0), t_oproj(j, 1)]
                return th

            # ---- main loop ----
            for t in phase1_thunks(0):
                t()
            def alloc_ctx():
                return ps.tile(
                    [HD + 1, 1024], fp32, tag="ctx", bufs=1, name="ctxm"
                )

            for ch in range(NCH):
                if ch > 0:
                    reserve.extend(outproj_thunks(ch - 1))
                if ch + 1 < NCH:
                    filler.extend(phase1_thunks(ch + 1))

                qc = ch
                nkt = 4 * (qc + 1)
                ctxm = alloc_ctx()
                pend = []  # deferred PV: (kt, pm, q0), depth 2
                for kt in range(nkt):
                    dd = kt - 4 * qc
                    q0 = 128 * dd if dd > 0 else 0
                    kch, kj = divmod(kt, 4)
                    ks = slice(kj * 128, kj * 128 + 128)
                    # scores^T [kl, h, q]: one PSUM bank per head so the
                    # (0,0)/(64,0) tile_position pair runs concurrently.
                    sm = ps.tile([128, 2, 512], fp32, tag="sm", bufs=2, name="sm")
                    for h in range(2):
                        hs = slice(HD * h, HD * h + HD)
                        nc.tensor.matmul(
                            sm[:, h, q0:512],
                            qk_all[hs, kch, 1, ks],
                            qk_all[hs, qc, 0, q0:512],
                            start=True, stop=True,
                            tile_position=(HD * h, 0),
                        )
                    pm = pm_pool.tile([128, 2, 512], bf16, tag="pm", bufs=4, name="pm")
                    nc.scalar.activation(
                        pm[:, :, q0:512], sm[:, :, q0:512], Exp, scale=SCALE
                    )
                    if dd >= 0:
                        nc.vector.tensor_mul(
                            pm[:, :, q0 : q0 + 128], pm[:, :, q0 : q0 + 128], tri2[:]
                        )
                    # P@V first (its exp has 2 iterations of slack, so it
                    # is nearly always ready), THEN fillers: a filler whose
                    # input is late must not block the ready PV in the
                    # strict-FIFO PE queue
                    if len(pend) >= 2:
                        _emit_pv(nc, ctxm, v_aug, *pend.pop(0), nkt)
                    pend.append((kt, pm, q0))
                    nf = len(filler)
                    if nf:
                        drain(-(-nf // (nkt - kt)))
                    else:
                        drain_reserve(1)
                for p in pend:
                    _emit_pv(nc, ctxm, v_aug, *p, nkt)
                pend.clear()
                for t in norm_thunks(qc, ctxm):
                    t()
                drain(len(filler))

            drain_reserve(len(reserve))
            for t in outproj_thunks(NCH - 1):
                t()

    nc.compile()
    return nc


def _emit_pv(nc, ctxm, v_aug, kt, pm, q0, nkt):
    start = kt == 0
    stop = kt == nkt - 1
    nc.tensor.matmul(
        ctxm[:, q0:512], v_aug[:, kt, 0 : HD + 1], pm[:, 0, q0:512],
        start=start, stop=stop,
    )
    nc.tensor.matmul(
        ctxm[:, 512 + q0 : 1024], v_aug[:, kt, HD + 1 : 2 * HD + 2],
        pm[:, 1, q0:512],
        start=start, stop=stop,
    )


_NC_CACHE = {}


def _get_nc(n=SEQ):
    if n not in _NC_CACHE:
        _NC_CACHE[n] = build_bass(n)
    return _NC_CACHE[n]


def make_in_maps(x, W_q, W_k, W_v, W_o):
    import ml_dtypes

    bf16 = ml_dtypes.bfloat16
    n = x.shape[-2]
    xT = np.asarray(x, dtype=np.float32).reshape(n, D).T  # [d, n]
    # x2[p, ch, dit, c] = xT[dit*128 + p, ch*512 + c]
    x2 = np.ascontiguousarray(
        xT.reshape(D // 128, 128, n // 512, 512).transpose(1, 2, 0, 3)
    ).astype(bf16)

    def wperm(W, s):
        # w2[p, dit, c] = W[dit*128 + p, s][c]
        Ws = np.asarray(W, np.float32)[:, s]
        return np.ascontiguousarray(
            Ws.reshape(D // 128, 128, DC).transpose(1, 0, 2)
        ).astype(bf16)

    in_maps = []
    for c in range(N_CORES):
        s = slice(c * DC, (c + 1) * DC)
        in_maps.append(
            {
                "x2": x2,
                "wq": wperm(W_q, s),
                "wk": wperm(W_k, s),
                "wv": wperm(W_v, s),
                "wo": np.ascontiguousarray(np.asarray(W_o, np.float32)[s, :]).astype(bf16),
            }
        )
    return in_maps


def kernel(x, W_q, W_k, W_v, W_o, b_o):
    from concourse import bass_utils

    x = np.asarray(x)
    b, n, _ = x.shape
    assert b == 1 and n == SEQ

    nc = _get_nc(n)
    in_maps = make_in_maps(x, W_q, W_k, W_v, W_o)
    res = bass_utils.run_bass_kernel_spmd(nc, in_maps, list(range(N_CORES)))
    acc = np.zeros((n, D), dtype=np.float64)
    for r in res.results:
        acc += r["out"].astype(np.float64)
    acc += np.asarray(b_o, np.float64)[None, :]
    return acc.astype(np.float32).reshape(1, n, D)
